# revision 16
# baseline (speedup 1.0000x reference)
"""Trainium2 Bass kernel for nn_BoundleAdjustment (2M observations).

Two launches on all 8 NeuronCores (observations data-parallel, M/8 per core):

Launch A (device): pose table (qx..qw) -> rotation matrices R = f(q/|q|)
as fp16 [128, 9, 32], all on the Vector engine (reciprocal_approx_fast
for the 2/|q|^2 scale; no activation-table load).  One input DMA, one
output DMA.

Host staging (indexing + dtype casts only): gathers the derived R table,
raw pose translations, and patch rows by poses_idx/patch_idx, and lays
per-observation records out as 19 fp16 planes [128, 19, 2048] per core
(partition-major so DMA slices map 1:1).

Launch B (device): 2 asymmetric column-chunks (1280, 768); per chunk 5
grouped input DMAs ordered by first use.  r = R*p + t with fp16
tensor_tensor on DVE (2x mode); squares/sqrts/sign/arctans on the
Scalar engine; divisions via reciprocal_approx_fast (fp32).
az uses the sign identity az = (pi/2)*sign(ry) - atan(rx/ry); ry gets
+1e-30 AFTER its final add (tensor_scalar) and the sqrts carry a 1e-30
bias so every reciprocal input is a positive normal float.  The three
residuals are fused: rng/az/el write one [128, 3, cc] tile, then one
subtract against the adjacent X/Y/Z planes and one multiply against a
stride-0-broadcast w plane.  GpSimd is deliberately unused: concurrent
Pool-engine tensor ops inflate DVE op latency ~2-4x (SBUF contention).
"""

import numpy as np

M = 2097152
NCORES = 8
N = M // NCORES          # 262144 obs per core
P = 128
COLS = N // P            # 2048
CHUNKS = [(0, 1280), (1280, 768)]
NCH = len(CHUNKS)
NPOSE = 4096
PC = NPOSE // P          # 32 cols for pose table
PI = float(np.pi)

# launch B plane order (by first use):
# 0:c1 1:c2 2:e 3:R00 4:R01 5:R02 6:tx | 7:R10 8:R11 9:R12 10:ty
# | 11:R20 12:R21 13:R22 14:tz | 15:X 16:Y 17:Z 18:w
NPB = 19

_CACHE = {}


def _build_posetab():
    import concourse.tile as tile
    from concourse import bacc, mybir

    nc = bacc.Bacc("TRN2", target_bir_lowering=False, debug=False,
                   num_devices=NCORES)
    f32 = mybir.dt.float32
    f16 = mybir.dt.float16
    OP = mybir.AluOpType
    AF = mybir.ActivationFunctionType
    q_d = nc.declare_dram_parameter("q", [P, 4, PC], f32, isOutput=False)
    r_d = nc.declare_dram_parameter("rtab", [P, 9, PC], f16, isOutput=True)

    with tile.TileContext(nc) as tc:
        with tc.tile_pool(name="pp", bufs=1) as pp:
            vec, act, gp = nc.vector, nc.scalar, nc.gpsimd
            qt = pp.tile([P, 4, PC], f32, tag="q")
            nc.sync.dma_start(qt[:], q_d[:, :, :])
            qx, qy, qz, qw = (qt[:, k, :] for k in range(4))
            ot = pp.tile([P, 9, PC], f16, tag="o")

            cnt = [0]

            def T(dt=f32):
                cnt[0] += 1
                return pp.tile([P, PC], dt, tag=f"t{cnt[0]}",
                               name=f"t{cnt[0]}")

            def tt(a, b, op, dt=f32, eng=vec):
                d = T(dt)
                eng.tensor_tensor(out=d[:], in0=a, in1=b, op=op)
                return d

            sq = {}
            for nm, srcap in (("xx", qx), ("yy", qy), ("zz", qz), ("ww", qw)):
                sq[nm] = tt(srcap, srcap, OP.mult)
            s01 = tt(sq["xx"][:], sq["yy"][:], OP.add)
            s23 = tt(sq["zz"][:], sq["ww"][:], OP.add)
            h0 = tt(s01[:], s23[:], OP.add)
            h = T()
            vec.tensor_scalar(out=h[:], in0=h0[:], scalar1=0.5, scalar2=None,
                              op0=OP.mult)          # |q|^2 / 2
            u2 = T()
            vec.reciprocal_approx_fast(out=u2[:], in_=h[:])   # 2/|q|^2

            prods = {}
            for nm, a, b in (("xy", qx, qy), ("xz", qx, qz), ("yz", qy, qz),
                             ("wx", qw, qx), ("wy", qw, qy), ("wz", qw, qz)):
                prods[nm] = tt(a, b, OP.mult)

            # Diagonal: R_ii = 1 - u2*(m1+m2)  (use full squares = 2*half^2)
            def diag(m1, m2, oi):
                a = tt(sq[m1][:], sq[m2][:], OP.add)     # m1^2+m2^2
                b = tt(a[:], u2[:], OP.mult)             # 2(m1^2+m2^2)/|q|^2
                vec.tensor_scalar(out=ot[:, oi, :], in0=b[:], scalar1=-1.0,
                                  scalar2=1.0, op0=OP.mult, op1=OP.add)

            def offd(m1, m2, op, oi):
                a = tt(prods[m1][:], prods[m2][:], op)
                vec.tensor_tensor(out=ot[:, oi, :], in0=a[:], in1=u2[:],
                                  op=OP.mult)

            diag("yy", "zz", 0); diag("xx", "zz", 4); diag("xx", "yy", 8)
            offd("xy", "wz", OP.subtract, 1); offd("xz", "wy", OP.add, 2)
            offd("xy", "wz", OP.add, 3); offd("yz", "wx", OP.subtract, 5)
            offd("xz", "wy", OP.subtract, 6); offd("yz", "wx", OP.add, 7)
            nc.sync.dma_start(r_d[:, :, :], ot[:])
    nc.finalize()
    return nc


def _build_main():
    import concourse.tile as tile
    from concourse import bacc, mybir

    nc = bacc.Bacc("TRN2", target_bir_lowering=False, debug=False,
                   num_devices=NCORES)
    f32 = mybir.dt.float32
    f16 = mybir.dt.float16
    OP = mybir.AluOpType
    AF = mybir.ActivationFunctionType
    in_d = nc.declare_dram_parameter("in", [P, NPB, COLS], f16, isOutput=False)
    out_d = nc.declare_dram_parameter("out", [P, 3, COLS], f16, isOutput=True)

    with tile.TileContext(nc) as tc:
        with tc.tile_pool(name="inp", bufs=2) as inp, \
             tc.tile_pool(name="tmp2", bufs=2) as tmp2, \
             tc.tile_pool(name="tmp1", bufs=1) as tmp1:
            vec, act = nc.vector, nc.scalar
            eps = tmp1.tile([P, 1], f32, tag="eps", name="eps")
            vec.memset(eps[:], 1e-30)

            st = [dict() for _ in range(NCH)]   # per-chunk value environment

            def TT(ch, tag, dt=f16, pool=tmp2):
                cc = CHUNKS[ch][1]
                return pool.tile([P, cc], dt, tag=tag, name=f"{tag}_{ch}")

            def tt(ch, a, b, op, d):
                vec.tensor_tensor(out=d[:], in0=a, in1=b, op=op)
                return d

            # --- stage 0: DMAs for chunk ch ---
            def dma_in(ch):
                off, cc = CHUNKS[ch]
                g0 = inp.tile([P, 4, cc], f16, tag="g0", name=f"g0_{ch}")
                g1 = inp.tile([P, 3, cc], f16, tag="g1", name=f"g1_{ch}")
                g2 = inp.tile([P, 4, cc], f16, tag="g2", name=f"g2_{ch}")
                g3 = inp.tile([P, 4, cc], f16, tag="g3", name=f"g3_{ch}")
                g4 = inp.tile([P, 4, cc], f16, tag="g4", name=f"g4_{ch}")
                nc.sync.dma_start(g0[:], in_d[:, 0:4, off:off + cc])
                nc.sync.dma_start(g1[:], in_d[:, 4:7, off:off + cc])
                nc.sync.dma_start(g2[:], in_d[:, 7:11, off:off + cc])
                nc.sync.dma_start(g3[:], in_d[:, 11:15, off:off + cc])
                nc.sync.dma_start(g4[:], in_d[:, 15:19, off:off + cc])
                s = st[ch]
                s["c1"], s["c2"] = g0[:, 0, :], g0[:, 1, :]
                s["e"] = g0[:, 3, :]
                s["R0"] = [g0[:, 2, :], g1[:, 0, :], g1[:, 1, :]]
                s["tx"] = g1[:, 2, :]
                s["R1"] = [g2[:, k, :] for k in range(3)]
                s["ty"] = g2[:, 3, :]
                s["R2"] = [g3[:, k, :] for k in range(3)]
                s["tz"] = g3[:, 3, :]
                s["g4"] = g4
                s["proj"] = tmp2.tile([P, 3, cc], f16, tag="proj",
                                      name=f"proj_{ch}")

            # --- stage 1: rotation rows (18 fp16 DVE ops) ---
            def rot(ch):
                s = st[ch]

                def row(R3, tc_, sfx, out_tag):
                    m0 = tt(ch, R3[0], s["c1"], OP.mult,
                            TT(ch, "m0" + sfx, pool=tmp1) if False else
                            tmp1.tile([P, CHUNKS[ch][1]], f16, tag="m0",
                                      name=f"m0{sfx}_{ch}", bufs=2))
                    m1 = tt(ch, R3[1], s["c2"], OP.mult,
                            tmp1.tile([P, CHUNKS[ch][1]], f16, tag="m1",
                                      name=f"m1{sfx}_{ch}", bufs=2))
                    tt(ch, m0[:], m1[:], OP.add, m0)
                    m2 = tt(ch, R3[2], s["e"], OP.mult,
                            tmp1.tile([P, CHUNKS[ch][1]], f16, tag="m2",
                                      name=f"m2{sfx}_{ch}", bufs=2))
                    tt(ch, m2[:], tc_, OP.add, m2)
                    return tt(ch, m0[:], m2[:], OP.add, TT(ch, out_tag))

                s["rx"] = row(s["R0"], s["tx"], "x", "rx")
                s["ry"] = row(s["R1"], s["ty"], "y", "ry")
                s["rz"] = row(s["R2"], s["tz"], "z", "rz")
                # guarded fp32 ry: epsilon AFTER the cancelling adds
                ry32 = TT(ch, "ry32", f32)
                vec.tensor_scalar(out=ry32[:], in0=s["ry"][:], scalar1=1e-30,
                                  scalar2=None, op0=OP.add)
                s["ry32"] = ry32

            # --- stage 2a: ACT squares + sign (sqrt-epoch table) ---
            def squares(ch):
                s = st[ch]
                for nm, src in (("sqx", s["rx"]), ("sqy", s["ry"]),
                                ("sqz", s["rz"])):
                    d = TT(ch, nm, pool=tmp1)
                    act.activation(d[:], src[:], AF.Square)
                    s[nm] = d
                sg = TT(ch, "sg")
                act.activation(sg[:], s["ry32"][:], AF.Sign)
                s["sg"] = sg

            # --- stage 2b: DVE sums of squares ---
            def sums(ch):
                s = st[ch]
                s["rho2"] = tt(ch, s["sqx"][:], s["sqy"][:], OP.add,
                               TT(ch, "rho2", pool=tmp1))
                s["r2"] = tt(ch, s["rho2"][:], s["sqz"][:], OP.add,
                             TT(ch, "r2", pool=tmp1))

            # --- stage 2c: ACT sqrts (sqrt-epoch table) ---
            def sqrts(ch):
                s = st[ch]
                rho = TT(ch, "rho", f32, tmp1)
                act.activation(rho[:], s["rho2"][:], AF.Sqrt, bias=eps[:])
                s["rho"] = rho
                act.activation(s["proj"][:, 0, :], s["r2"][:], AF.Sqrt,
                               bias=eps[:])

            # --- stage 3: DVE reciprocals + atan args + rng residual ---
            def recips(ch):
                s = st[ch]
                irho = TT(ch, "irho", f32, tmp1)
                vec.reciprocal_approx_fast(out=irho[:], in_=s["rho"][:])
                iry = TT(ch, "iry", f32, tmp1)
                vec.reciprocal_approx_fast(out=iry[:], in_=s["ry32"][:])
                s["u"] = tt(ch, s["rx"][:], iry[:], OP.mult, TT(ch, "u", f32))
                s["v"] = tt(ch, s["rz"][:], irho[:], OP.mult, TT(ch, "v", f32))

            # --- stage 4: ACT atans (arctan-epoch table) ---
            def atans(ch):
                s = st[ch]
                a16 = TT(ch, "a16", pool=tmp1)
                act.activation(a16[:], s["u"][:], AF.Arctan)
                s["a16"] = a16
                el16 = s["proj"][:, 2, :]
                act.activation(el16, s["v"][:], AF.Arctan)

            # --- stage 5: DVE az + fused 3-component residual + out DMA ---
            def resid(ch):
                s = st[ch]
                off, cc = CHUNKS[ch]
                vec.scalar_tensor_tensor(out=s["proj"][:, 1, :],
                                         in0=s["sg"][:], scalar=PI / 2.0,
                                         in1=s["a16"][:],
                                         op0=OP.mult, op1=OP.subtract)
                vec.tensor_tensor(out=s["proj"][:], in0=s["proj"][:],
                                  in1=s["g4"][:, 0:3, :], op=OP.subtract)
                wb = s["g4"][:, 3:4, :].to_broadcast([P, 3, cc])
                vec.tensor_tensor(out=s["proj"][:], in0=s["proj"][:], in1=wb,
                                  op=OP.mult)
                nc.sync.dma_start(out_d[:, :, off:off + cc], s["proj"][:])

            # Emission order: DVE fills ACT-latency gaps with the other
            # chunk's work; all sqrt-table ACT ops precede all arctan ops
            # (exactly one activation-table swap).
            dma_in(0); dma_in(1)
            rot(0)
            squares(0)
            rot(1)
            sums(0)
            sqrts(0)
            squares(1)
            recips(0)
            sums(1)
            sqrts(1)
            recips(1)
            atans(0)
            resid(0)
            atans(1)
            resid(1)
    nc.finalize()
    return nc


def _get(name, builder):
    if name not in _CACHE:
        _CACHE[name] = builder()
    return _CACHE[name]


def _stage_q(poses):
    # pose n = p*PC + c  ->  q_planes[p, :, c]
    return np.ascontiguousarray(
        poses[:, 3:7].reshape(P, PC, 4).transpose(0, 2, 1)).astype(np.float32)


def _rtab_from_result(rt):
    # rt: [P, 9, PC] fp16 -> [NPOSE, 9]
    return np.ascontiguousarray(
        np.asarray(rt).transpose(0, 2, 1).reshape(NPOSE, 9))


def _stage_planes(rtab, poses, patch_coords, elevation_angle, pid, qid,
                  target_coords, weights):
    r9 = rtab[pid]                                        # [M, 9] fp16
    t3 = poses[pid, 0:3].astype(np.float16)               # [M, 3]
    pc = patch_coords[qid].astype(np.float16)
    ea = elevation_angle[qid].astype(np.float16)
    big = np.concatenate(
        [pc, r9[:, 0:1], ea, r9[:, 1:3], t3[:, 0:1],
         r9[:, 3:6], t3[:, 1:2],
         r9[:, 6:9], t3[:, 2:3],
         target_coords.astype(np.float16),
         weights.astype(np.float16)], axis=1)             # [M, 19]
    return np.ascontiguousarray(
        big.reshape(NCORES, P, COLS, NPB).transpose(0, 1, 3, 2))


def _unstage_out(res_list):
    out = np.stack([np.asarray(res_list[c]["out"]) for c in range(NCORES)])
    return np.ascontiguousarray(
        out.transpose(0, 1, 3, 2).reshape(M, 3)).astype(np.float32)


def kernel(poses, patch_coords, elevation_angle, poses_idx, patch_idx,
           target_coords, weights):
    from concourse.bass_utils import run_bass_kernel_spmd

    poses = np.asarray(poses, dtype=np.float32)
    patch_coords = np.asarray(patch_coords, dtype=np.float32)
    elevation_angle = np.asarray(elevation_angle, dtype=np.float32)
    target_coords = np.asarray(target_coords, dtype=np.float32)
    weights = np.asarray(weights, dtype=np.float32)
    pid = np.asarray(poses_idx).astype(np.int64)
    qid = np.asarray(patch_idx).astype(np.int64)

    # ---- launch A: pose table -> fp16 rotation matrices (device) ----
    q_planes = _stage_q(poses)
    ncA = _get("A", _build_posetab)
    resA = run_bass_kernel_spmd(ncA, [{"q": q_planes} for _ in range(NCORES)],
                                list(range(NCORES)))
    rtab = _rtab_from_result(resA.results[0]["rtab"])

    # ---- host: gather / per-obs staging (indexing + dtype cast only) ----
    big = _stage_planes(rtab, poses, patch_coords, elevation_angle, pid, qid,
                        target_coords, weights)

    # ---- launch B: streaming rotate+polar+residual ----
    ncB = _get("B", _build_main)
    resB = run_bass_kernel_spmd(ncB, [{"in": big[c]} for c in range(NCORES)],
                                list(range(NCORES)))
    return _unstage_out(resB.results)
